# revision 14
# baseline (speedup 1.0000x reference)
"""Greedy CTC decoder on Trainium2 (Bass/Tile), sharded over 8 NeuronCores.

Input : emission [65536, 512] float32 (full, unsharded)
Output: (index [65536] int32, keep [65536] bool) matching the reference:
    index = argmax(emission, axis=-1)
    char  = index - 1 (blank 0 -> -1)
    keep  = (char != prev_char) & (char != -1)
          = (index != prev_index) & (index != 0),  prev of t=0 is a sentinel

Sharding: timestep axis T split across 8 cores (8192 rows each). Inside a
core, partition p owns the 64 consecutive timesteps p*64..p*64+63; row
column j of a chunk is one timestep per partition.

The kernel is HBM-bound (~47us/core for the 16MiB emission read), so the
whole decode is ONE custom DVE instruction per row, registered at import
time, that streams the row through BOTH SBUF read ports (in0 = even
elements, in1 = odd elements, stride 2) and folds pairs on the fly:

    m2    = max(a, b)                     # one vocab PAIR per cycle
    body  = select(eq(m2, running_max(m2)), Idx, -FLT_MAX)
    accum = MAX  -> last pair index whose max equals the row max

256 cycles + ~150 overhead per 512-wide row, ~0.56us/row incl. the
accumulator readout - half the cost of the stock tensor_reduce +
FIND_INDEX8 pair (previous bottleneck), and the index needs no needles
or collision repair. The host resolves the within-pair bit with two
vectorized gathers and computes the repeat-collapse mask (O(T) numpy).
Ties of the row max across pairs resolve to the LAST pair instead of
jnp.argmax's first occurrence: 3 rows in 65536 for these inputs.
"""

import numpy as np

import concourse.bacc as bacc
import concourse.mybir as mybir
import concourse.dve_ops as dve_ops
from concourse.dve_spec import (Spec, Src0, Src1, Idx, MaxNeg, AluOp,
                                scan, eq, select, maxx, lower)
from concourse.dve_uop import DveOpSpec
from concourse.tile import TileContext
from concourse.bass_utils import run_bass_kernel_spmd

N_CORES = 8
T_FULL = 65536
V = 512
P = 128
T_SHARD = T_FULL // N_CORES          # 8192
JPP = T_SHARD // P                   # 64 timesteps per partition
# DMA chunk sizes (timesteps per partition per DMA). The DMA engines cap at
# ~26 GB/s x 16 = ~416 GB/s/core, and each chunk costs ~3.5us of serial
# descriptor dispatch (128 descriptors, one per partition) on its issue
# queue; chunks alternate between the two hardware-DGE queues (SP and
# Activation) to parallelize dispatch. Small chunks at the ends for
# pipeline fill/drain, 32KB-per-partition descriptors in the middle.
CHUNKS = [4, 4] + [8] * 6 + [4, 2, 2]
SPLIT = 56

_prog_cache = {}


def _register_argmax_op():
    """Register the dual-stream pair-argmax DVE op (idempotent)."""
    name = "ARGMAX_PAIR2_ANT"
    if name in dve_ops._SUB_OPCODE_FOR_NAME:
        for op in dve_ops.OPS:
            if op.name == name:
                return op
    m2 = maxx(Src0, Src1)
    body = select(eq(m2, scan(AluOp.MAX, m2)), Idx, MaxNeg)

    def _ref(in0, in1):
        m2 = np.maximum(in0, in1)
        r = np.maximum.accumulate(m2, axis=-1)
        o = np.where(m2 == r,
                     np.arange(m2.shape[-1], dtype=np.float32),
                     -np.finfo(np.float32).max)
        return o, o.max(axis=-1, keepdims=True)

    spec = Spec(body=body, accum=AluOp.MAX, reference=_ref)
    row = dve_ops._CUSTOM_DVE_ROW_BASE + len(dve_ops.OPS)
    assert row < 0x20
    shas = {}
    for ver in ("v3", "v4"):
        try:
            ds = DveOpSpec(name=name, opcode=row, uops=lower(spec, ver=ver),
                           rd1_en=True)
            shas[ver] = ds.sha(ver)
        except Exception:
            pass
    op = dve_ops.DveOp(name, spec, subdim=False, uops_sha=shas)
    dve_ops.OPS.append(op)
    dve_ops.CUSTOM_DVE_SPECS[name] = spec
    dve_ops._SUB_OPCODE_FOR_NAME[name] = row
    return op


def _build():
    op = _register_argmax_op()
    nc = bacc.Bacc(None, target_bir_lowering=False)

    em_h = nc.dram_tensor("emission", [T_SHARD, V], mybir.dt.float32,
                          kind="ExternalInput")
    idx_h = nc.dram_tensor("idx_out", [T_SHARD], mybir.dt.float32,
                           kind="ExternalOutput")

    # [T_SHARD, V] -> [P, JPP, V]: partition p holds rows p*JPP .. p*JPP+JPP-1
    em3 = em_h[:, :].rearrange("(p j) v -> p j v", p=P)
    idx_out2 = idx_h[:].rearrange("(p j) -> p j", p=P)

    with TileContext(nc) as tc:
        with (
            tc.tile_pool(name="acc", bufs=1) as acc_pool,
        ):
            idxp = acc_pool.tile([P, JPP], mybir.dt.float32)
            scratch = acc_pool.tile([P, V // 2], mybir.dt.float32)

            # the whole shard is 128KB/partition - it fits in SBUF, so every
            # chunk gets its own persistent tile and all input DMAs can be
            # issued immediately (no buffer ring coupling DMA to compute)
            j = 0
            for c, n in enumerate(CHUNKS):
                tile = acc_pool.tile([P, n, V], mybir.dt.float32,
                                     name=f"chunk{c}", tag=f"chunk{c}")
                q = nc.sync if c % 2 == 0 else nc.scalar
                q.dma_start(out=tile[:, :, :], in_=em3[:, j:j + n, :])
                t4 = tile[:, :, :].rearrange("p a (v two) -> p a v two", two=2)
                for k in range(n):
                    nc.vector._custom_dve(op, out=scratch[:, :],
                                          in0=t4[:, k, :, 0],
                                          in1=t4[:, k, :, 1],
                                          accum_out=idxp[:, j + k:j + k + 1])
                j += n
                if j == SPLIT:
                    nc.sync.dma_start(out=idx_out2[:, 0:SPLIT],
                                      in_=idxp[:, 0:SPLIT])
            nc.sync.dma_start(out=idx_out2[:, SPLIT:JPP],
                              in_=idxp[:, SPLIT:JPP])

    nc.compile()
    return nc


def _get_prog():
    if "nc" not in _prog_cache:
        _prog_cache["nc"] = _build()
    return _prog_cache["nc"]


def run_sharded(emission: np.ndarray, **spmd_kwargs):
    """Run the SPMD kernel; returns (idx int32 [T], keep bool [T], results)."""
    emission = np.ascontiguousarray(np.asarray(emission, dtype=np.float32))
    assert emission.shape == (T_FULL, V), emission.shape
    nc = _get_prog()
    in_maps = [
        {"emission": np.ascontiguousarray(emission[c * T_SHARD:(c + 1) * T_SHARD])}
        for c in range(N_CORES)
    ]
    res = run_bass_kernel_spmd(nc, in_maps, list(range(N_CORES)), **spmd_kwargs)
    rawp = np.concatenate([res.results[c]["idx_out"] for c in range(N_CORES)])
    p2 = rawp.astype(np.int64) * 2
    t = np.arange(T_FULL)
    # within-pair resolution: first occurrence wins on equality, matching
    # jnp.argmax
    idx = (p2 + (emission[t, p2 + 1] > emission[t, p2])).astype(np.int32)
    prev = np.concatenate([np.full(1, -1, dtype=np.int32), idx[:-1]])
    keep = (idx != prev) & (idx != 0)
    return idx, keep, res


def kernel(emission: np.ndarray):
    idx, keep, _ = run_sharded(emission)
    return idx, keep


# revision 15
# speedup vs baseline: 1.2232x; 1.2232x over previous
"""Greedy CTC decoder on Trainium2 (Bass/Tile), sharded over 8 NeuronCores.

Input : emission [65536, 512] float32 (full, unsharded)
Output: (index [65536] int32, keep [65536] bool) matching the reference:
    index = argmax(emission, axis=-1)
    char  = index - 1 (blank 0 -> -1)
    keep  = (char != prev_char) & (char != -1)
          = (index != prev_index) & (index != 0),  prev of t=0 is a sentinel

Sharding: timestep axis T split across 8 cores (8192 rows each). Inside a
core, partition p owns the 64 consecutive timesteps p*64..p*64+63; row
column j of a chunk is one timestep per partition.

The kernel is HBM-bound (~47us/core for the 16MiB emission read), so the
whole decode is ONE custom DVE instruction per row, registered at import
time, that streams the row through BOTH SBUF read ports (in0 = even
elements, in1 = odd elements, stride 2) and folds pairs on the fly:

    m2    = max(a, b)                     # one vocab PAIR per cycle
    body  = select(eq(m2, running_max(m2)), Idx, -FLT_MAX)
    accum = MAX  -> last pair index whose max equals the row max

256 cycles + ~150 overhead per 512-wide row, ~0.56us/row incl. the
accumulator readout - half the cost of the stock tensor_reduce +
FIND_INDEX8 pair (previous bottleneck), and the index needs no needles
or collision repair. The host resolves the within-pair bit with two
vectorized gathers and computes the repeat-collapse mask (O(T) numpy).
Ties of the row max across pairs resolve to the LAST pair instead of
jnp.argmax's first occurrence: 3 rows in 65536 for these inputs.
"""

import numpy as np

import concourse.bacc as bacc
import concourse.mybir as mybir
import concourse.dve_ops as dve_ops
from concourse.dve_spec import (Spec, Src0, Src1, Idx, MaxNeg, AluOp,
                                scan, eq, select, maxx, lower)
from concourse.dve_uop import DveOpSpec
from concourse.tile import TileContext
from concourse.bass_utils import run_bass_kernel_spmd

N_CORES = 8
T_FULL = 65536
V = 512
P = 128
T_SHARD = T_FULL // N_CORES          # 8192
JPP = T_SHARD // P                   # 64 timesteps per partition
# DMA chunk sizes (timesteps per partition per DMA). The DMA engines cap at
# ~26 GB/s x 16 = ~416 GB/s/core, and each chunk costs ~3.5us of serial
# descriptor dispatch (128 descriptors, one per partition) on its issue
# queue; chunks alternate between the two hardware-DGE queues (SP and
# Activation) to parallelize dispatch. Small chunks at the ends for
# pipeline fill/drain, 32KB-per-partition descriptors in the middle.
CHUNKS = [4, 4] + [8] * 6 + [4, 2, 2]
SPLIT = 56

_prog_cache = {}


def _register_argmax_op():
    """Register the dual-stream pair-argmax DVE op (idempotent)."""
    name = "ARGMAX_PAIR2_ANT"
    if name in dve_ops._SUB_OPCODE_FOR_NAME:
        for op in dve_ops.OPS:
            if op.name == name:
                return op
    m2 = maxx(Src0, Src1)
    body = select(eq(m2, scan(AluOp.MAX, m2)), Idx, MaxNeg)

    def _ref(in0, in1):
        m2 = np.maximum(in0, in1)
        r = np.maximum.accumulate(m2, axis=-1)
        o = np.where(m2 == r,
                     np.arange(m2.shape[-1], dtype=np.float32),
                     -np.finfo(np.float32).max)
        return o, o.max(axis=-1, keepdims=True)

    spec = Spec(body=body, accum=AluOp.MAX, reference=_ref)
    row = dve_ops._CUSTOM_DVE_ROW_BASE + len(dve_ops.OPS)
    assert row < 0x20
    shas = {}
    for ver in ("v3", "v4"):
        try:
            ds = DveOpSpec(name=name, opcode=row, uops=lower(spec, ver=ver),
                           rd1_en=True)
            shas[ver] = ds.sha(ver)
        except Exception:
            pass
    op = dve_ops.DveOp(name, spec, subdim=False, uops_sha=shas)
    dve_ops.OPS.append(op)
    dve_ops.CUSTOM_DVE_SPECS[name] = spec
    dve_ops._SUB_OPCODE_FOR_NAME[name] = row
    return op


def _build():
    op = _register_argmax_op()
    nc = bacc.Bacc(None, target_bir_lowering=False)

    em_h = nc.dram_tensor("emission", [T_SHARD, V], mybir.dt.float32,
                          kind="ExternalInput")
    idx_h = nc.dram_tensor("idx_out", [T_SHARD], mybir.dt.float32,
                           kind="ExternalOutput")

    # [T_SHARD, V] -> [P, JPP, V]: partition p holds rows p*JPP .. p*JPP+JPP-1
    em3 = em_h[:, :].rearrange("(p j) v -> p j v", p=P)
    idx_out2 = idx_h[:].rearrange("(p j) -> p j", p=P)

    with TileContext(nc) as tc:
        with (
            tc.tile_pool(name="acc", bufs=1) as acc_pool,
        ):
            idxp = acc_pool.tile([P, JPP], mybir.dt.float32)
            scratch = acc_pool.tile([P, V // 2], mybir.dt.float32)

            # the whole shard is 128KB/partition - it fits in SBUF, so every
            # chunk gets its own persistent tile and all input DMAs can be
            # issued immediately (no buffer ring coupling DMA to compute)
            j = 0
            for c, n in enumerate(CHUNKS):
                tile = acc_pool.tile([P, n, V], mybir.dt.float32,
                                     name=f"chunk{c}", tag=f"chunk{c}")
                nc.sync.dma_start(out=tile[:, :, :], in_=em3[:, j:j + n, :])
                t4 = tile[:, :, :].rearrange("p a (v two) -> p a v two", two=2)
                for k in range(n):
                    nc.vector._custom_dve(op, out=scratch[:, :],
                                          in0=t4[:, k, :, 0],
                                          in1=t4[:, k, :, 1],
                                          accum_out=idxp[:, j + k:j + k + 1])
                j += n
                if j == SPLIT:
                    nc.sync.dma_start(out=idx_out2[:, 0:SPLIT],
                                      in_=idxp[:, 0:SPLIT])
            nc.sync.dma_start(out=idx_out2[:, SPLIT:JPP],
                              in_=idxp[:, SPLIT:JPP])

    nc.compile()
    return nc


def _get_prog():
    if "nc" not in _prog_cache:
        _prog_cache["nc"] = _build()
    return _prog_cache["nc"]


def run_sharded(emission: np.ndarray, **spmd_kwargs):
    """Run the SPMD kernel; returns (idx int32 [T], keep bool [T], results)."""
    emission = np.ascontiguousarray(np.asarray(emission, dtype=np.float32))
    assert emission.shape == (T_FULL, V), emission.shape
    nc = _get_prog()
    in_maps = [
        {"emission": np.ascontiguousarray(emission[c * T_SHARD:(c + 1) * T_SHARD])}
        for c in range(N_CORES)
    ]
    res = run_bass_kernel_spmd(nc, in_maps, list(range(N_CORES)), **spmd_kwargs)
    rawp = np.concatenate([res.results[c]["idx_out"] for c in range(N_CORES)])
    p2 = rawp.astype(np.int64) * 2
    t = np.arange(T_FULL)
    # within-pair resolution: first occurrence wins on equality, matching
    # jnp.argmax
    idx = (p2 + (emission[t, p2 + 1] > emission[t, p2])).astype(np.int32)
    prev = np.concatenate([np.full(1, -1, dtype=np.int32), idx[:-1]])
    keep = (idx != prev) & (idx != 0)
    return idx, keep, res


def kernel(emission: np.ndarray):
    idx, keep, _ = run_sharded(emission)
    return idx, keep
